# revision 13
# baseline (speedup 1.0000x reference)
"""Cross-attention kernel for Trainium2 (8 NeuronCores).

Problem: B=4, SQ=SKV=2048, D=512, H=8, DH=64 cross-attention with
LayerNorm on both streams, returning (out [B,SQ,D], attn [B,H,SQ,SKV]).

Sharding: core c handles batch b = c//2 and heads [4*(c%2), 4*(c%2)+4).
Each core computes LN + QKV projections for its (batch, head-group),
full attention for its 4 heads (including the 64MB attn-probability
slab), and a partial out-projection. Host sums the two half-head
partials per batch and adds the bias.

Device dataflow per core:
  - LN in natural layout (bn_stats/bn_aggr), PE-transpose to get
    xn^T/yn^T with D on partitions.
  - q^T/k^T [64, 2048] per head and V [2048, 4*65] (with a ones column
    per head) via f32r matmuls.
  - Phase A per head: S^T = k^T.T @ q^T tiles, exp on ScalarE, then
    O^T = [V|1]^T @ exp(S^T) accumulated over Skv. Row 64 of O^T is the
    softmax denominator for free. O^T is normalized per-column via a
    PE-broadcast reciprocal and stored (f32r) for the out-projection.
  - Phase B per head: S = q^T.T @ k^T natural tiles; attention
    probabilities emitted in ONE ScalarE op per row-block:
    A = exp(S*scale - ln d) using the per-partition bias input of the
    activation (d transposed from phase A via PE).
  - Out-projection from the stacked normalized O^T chunks.

All heavy matmuls run in float32r (TF32-like, 4x faster than fp32 on
the PE; ~2e-4 relative error).
"""

import sys

if "/opt/trn_rl_repo" not in sys.path:
    sys.path.insert(0, "/opt/trn_rl_repo")

import numpy as np

import concourse.bacc as bacc
import concourse.mybir as mybir
from concourse.tile import TileContext
from concourse.bass_utils import run_bass_kernel_spmd
from concourse.masks import make_identity

F32 = mybir.dt.float32
F32R = mybir.dt.float32r
AF = mybir.ActivationFunctionType
OP = mybir.AluOpType

B, SQ, SKV, D, H, DH = 4, 2048, 2048, 512, 8, 64
HPC = 4          # heads per core
N_CORES = 8
EPS = 1e-5
SCALE = DH ** -0.5
DC = D // 128    # 4 D-chunks of 128


def build_kernel():
    nc = bacc.Bacc()

    x_b = nc.declare_dram_parameter("x_b", [SQ, D], F32, isOutput=False)
    y_b = nc.declare_dram_parameter("y_b", [SKV, D], F32, isOutput=False)
    wq = nc.declare_dram_parameter("wq", [D, HPC * DH], F32, isOutput=False)
    wk = nc.declare_dram_parameter("wk", [D, HPC * DH], F32, isOutput=False)
    wv = nc.declare_dram_parameter("wv", [D, HPC * DH], F32, isOutput=False)
    wo = nc.declare_dram_parameter("wo", [HPC * DH, D], F32, isOutput=False)
    gq = nc.declare_dram_parameter("gq", [1, D], F32, isOutput=False)
    bq = nc.declare_dram_parameter("bq", [1, D], F32, isOutput=False)
    gkv = nc.declare_dram_parameter("gkv", [1, D], F32, isOutput=False)
    bkv = nc.declare_dram_parameter("bkv", [1, D], F32, isOutput=False)

    attn_p = nc.declare_dram_parameter("attn_p", [HPC, SQ, SKV], F32, isOutput=True)
    out_p = nc.declare_dram_parameter("out_p", [SQ, D], F32, isOutput=True)

    with TileContext(nc) as tc:
        with (
            tc.tile_pool(name="const", bufs=1) as cpool,
            tc.tile_pool(name="persist", bufs=1) as ppool,
            tc.tile_pool(name="work", bufs=3) as wpool,
            tc.tile_pool(name="rows", bufs=1) as rpool,
        ):
            # ---- constants ----
            ident = cpool.tile([128, 128], F32)
            make_identity(nc, ident[:, :])
            epst = cpool.tile([128, 1], F32)
            nc.gpsimd.memset(epst[:, :], EPS)
            ones_row = cpool.tile([1, 128], F32)
            nc.gpsimd.memset(ones_row[:, :], 1.0)
            ones_col = cpool.tile([128, 1], F32)
            nc.gpsimd.memset(ones_col[:, :], 1.0)
            ones_row_r = cpool.tile([1, 128], F32R)
            nc.vector.tensor_copy(ones_row_r[:, :], ones_row[:, :])

            # ---- ln scale/shift broadcast to [128, D] ----
            gb_rows = []
            for idx, t in enumerate((gq, bq, gkv, bkv)):
                row = cpool.tile([1, D], F32, tag=f"gbrow{idx}")
                nc.sync.dma_start(out=row[:, :], in_=t[:, :])
                gb_rows.append(row)
            gbb = []  # Gq, Bq, Gkv, Bkv broadcast tiles
            with tc.tile_pool(name="ps_init", bufs=2, space="PSUM") as psi:
                for idx in range(4):
                    pb = psi.tile([128, D], F32, tag="gb")
                    # fp32 matmul (K=1): broadcast row to 128 partitions
                    nc.tensor.matmul(pb[:, :], ones_row[0:1, :], gb_rows[idx][:, :],
                                     start=True, stop=True)
                    t = cpool.tile([128, D], F32, tag=f"gbb{idx}")
                    nc.vector.tensor_copy(t[:, :], pb[:, :])
                    gbb.append(t)

            # ---- weights ----
            wq_r = ppool.tile([128, DC, HPC * DH], F32R)
            wk_r = ppool.tile([128, DC, HPC * DH], F32R)
            wv_r = ppool.tile([128, DC, HPC * DH], F32R)
            wo_r = ppool.tile([128, 2, D], F32R)
            for (dram, tile_r, nchunk) in ((wq, wq_r, DC), (wk, wk_r, DC),
                                           (wv, wv_r, DC), (wo, wo_r, 2)):
                stage = wpool.tile([128, nchunk, dram.shape[1]], F32, tag="wstage", bufs=1)
                nc.sync.dma_start(
                    out=stage[:, :, :],
                    in_=dram.rearrange("(c p) n -> p c n", p=128))
                nc.vector.tensor_copy(tile_r[:, :, :], stage[:, :, :])

            # ---- LN + transpose helper ----
            def ln_transpose(src_dram, G, Bt, dst_pool, tagbase):
                """LayerNorm src [2048, 512] then PE-transpose into 4 chunk
                tiles [128, 2048] (f32r), D on partitions."""
                chunks = [dst_pool.tile([128, SQ], F32R, tag=f"{tagbase}{c}", name=f"{tagbase}{c}")
                          for c in range(DC)]
                with tc.tile_pool(name=f"ps_{tagbase}", bufs=4, space="PSUM") as pst:
                    for i in range(SQ // 128):
                        t = wpool.tile([128, D], F32, tag="ln_in", bufs=2)
                        nc.sync.dma_start(out=t[:, :],
                                          in_=src_dram[i * 128:(i + 1) * 128, :])
                        st = wpool.tile([128, 6], F32, tag="ln_st")
                        nc.vector.bn_stats(st[:, :], t[:, :])
                        ag = wpool.tile([128, 2], F32, tag="ln_ag")
                        nc.vector.bn_aggr(ag[:, :], st[:, :])
                        std = wpool.tile([128, 1], F32, tag="ln_std")
                        nc.scalar.activation(std[:, :], ag[:, 1:2], AF.Sqrt,
                                             bias=epst[:, 0:1])
                        rstd = wpool.tile([128, 1], F32, tag="ln_rstd")
                        nc.vector.reciprocal(rstd[:, :], std[:, :])
                        xc = wpool.tile([128, D], F32, tag="ln_xc", bufs=2)
                        nc.vector.tensor_scalar(
                            xc[:, :], t[:, :], ag[:, 0:1], rstd[:, :],
                            op0=OP.subtract, op1=OP.mult)
                        xg = wpool.tile([128, D], F32, tag="ln_xg", bufs=2)
                        nc.vector.tensor_tensor(out=xg[:, :], in0=xc[:, :],
                                                in1=G[:, :], op=OP.mult)
                        xn = wpool.tile([128, D], F32, tag="ln_xn", bufs=2)
                        nc.vector.tensor_tensor(out=xn[:, :], in0=xg[:, :],
                                                in1=Bt[:, :], op=OP.add)
                        for c in range(DC):
                            pt = pst.tile([128, 128], F32, tag="tr")
                            nc.tensor.transpose(pt[:, :],
                                                xn[:, c * 128:(c + 1) * 128],
                                                ident[:, :])
                            nc.vector.tensor_copy(
                                chunks[c][:, i * 128:(i + 1) * 128], pt[:, :])
                return chunks

            # ---- stream x: LN -> xT -> q^T ----
            q2T = [ppool.tile([128, SQ], F32R, tag=f"q2T{g}", name=f"q2T{g}")
                   for g in range(HPC // 2)]
            k2T = [ppool.tile([128, SKV], F32R, tag=f"k2T{g}", name=f"k2T{g}")
                   for g in range(HPC // 2)]
            qT = [q2T[h // 2][(h % 2) * DH:(h % 2 + 1) * DH, :] for h in range(HPC)]
            kT = [k2T[h // 2][(h % 2) * DH:(h % 2 + 1) * DH, :] for h in range(HPC)]
            # V plus ones column, per skv-chunk: [128, 16, 4*65]
            vp = ppool.tile([128, SKV // 128, HPC * (DH + 1)], F32R)

            with tc.tile_pool(name="xT_pool", bufs=1) as xpool:
                xT = ln_transpose(x_b, gbb[0], gbb[1], xpool, "xT")
                with tc.tile_pool(name="ps_q", bufs=2, space="PSUM") as psq:
                    for h in range(HPC):
                        for j in range(SQ // 512):
                            pq = psq.tile([DH, 512], F32, tag="pq")
                            for c in range(DC):
                                nc.tensor.matmul(
                                    pq[:, :],
                                    wq_r[:, c, h * DH:(h + 1) * DH],
                                    xT[c][:, j * 512:(j + 1) * 512],
                                    start=(c == 0), stop=(c == DC - 1))
                            nc.vector.tensor_copy(
                                qT[h][:, j * 512:(j + 1) * 512], pq[:, :])

            # ---- stream y: LN -> yT -> k^T, V ----
            with tc.tile_pool(name="yT_pool", bufs=1) as ypool:
                yT = ln_transpose(y_b, gbb[2], gbb[3], ypool, "yT")
                with tc.tile_pool(name="ps_k", bufs=2, space="PSUM") as psk:
                    for h in range(HPC):
                        for j in range(SKV // 512):
                            pk = psk.tile([DH, 512], F32, tag="pk")
                            for c in range(DC):
                                nc.tensor.matmul(
                                    pk[:, :],
                                    wk_r[:, c, h * DH:(h + 1) * DH],
                                    yT[c][:, j * 512:(j + 1) * 512],
                                    start=(c == 0), stop=(c == DC - 1))
                            nc.vector.tensor_copy(
                                kT[h][:, j * 512:(j + 1) * 512], pk[:, :])
                    for m in range(SKV // 128):
                        pv = psk.tile([128, HPC * DH], F32, tag="pv")
                        for c in range(DC):
                            nc.tensor.matmul(
                                pv[:, :],
                                yT[c][:, m * 128:(m + 1) * 128],
                                wv_r[:, c, :],
                                start=(c == 0), stop=(c == DC - 1))
                        for h in range(HPC):
                            nc.vector.tensor_copy(
                                vp[:, m, h * 65:h * 65 + DH],
                                pv[:, h * DH:(h + 1) * DH])
                            nc.vector.tensor_copy(
                                vp[:, m, h * 65 + DH:h * 65 + DH + 1],
                                ones_col[:, :])

            # normalized O^T stacked in two 128-row chunks for the out-proj
            ost = [ppool.tile([128, SQ], F32R, tag=f"ost{c}", name=f"ost{c}") for c in range(2)]
            # per-head, per-row-block -ln(denominator): [128, 16]
            nld = [ppool.tile([128, SQ // 128], F32, tag=f"nld{h}", name=f"nld{h}")
                   for h in range(HPC)]

            n_m = SKV // 128  # 16 skv chunks

            apool_cm = tc.tile_pool(name="attn_work", bufs=2)
            apool = apool_cm.__enter__()
            for h in range(HPC):
                # ================= phase A: S^T, AV, denominators ============
                with tc.tile_pool(name=f"psA{h}", bufs=1, space="PSUM") as psa:
                    av = [psa.tile([65, 512], F32, tag=f"av{j}", name=f"av{j}")
                          for j in range(4)]
                    den_row = rpool.tile([1, SQ], F32, tag="den_row")
                    for half in range(2):
                        for m in range(n_m):
                            ps_st = psa.tile([128, 1024], F32, tag="st", bufs=2)
                            for jj in range(2):
                                nc.tensor.matmul(
                                    ps_st[:, jj * 512:(jj + 1) * 512],
                                    kT[h][:, m * 128:(m + 1) * 128],
                                    qT[h][:, half * 1024 + jj * 512:
                                           half * 1024 + (jj + 1) * 512],
                                    start=True, stop=True)
                            eT = apool.tile([128, 1024], F32R, tag="eT", bufs=3)
                            nc.scalar.activation(eT[:, :], ps_st[:, :], AF.Exp,
                                                 scale=SCALE)
                            for jj in range(2):
                                nc.tensor.matmul(
                                    av[half * 2 + jj][:, :],
                                    vp[:, m, h * 65:(h + 1) * 65],
                                    eT[:, jj * 512:(jj + 1) * 512],
                                    start=(m == 0), stop=(m == n_m - 1))
                    # normalize O^T, collect denominators
                    for j in range(4):
                        nc.vector.tensor_copy(den_row[0:1, j * 512:(j + 1) * 512],
                                              av[j][DH:DH + 1, :])
                        recip = rpool.tile([1, 512], F32, tag="recip")
                        nc.vector.reciprocal(recip[:, :], av[j][DH:DH + 1, :])
                        recip_r = rpool.tile([1, 512], F32R, tag="recip_r")
                        nc.vector.tensor_copy(recip_r[:, :], recip[:, :])
                        pbc = psa.tile([DH, 512], F32, tag="st", bufs=2, name="pbc")
                        nc.tensor.matmul(pbc[:, :], ones_row_r[0:1, 0:DH],
                                         recip_r[:, :], start=True, stop=True)
                        bcs = wpool.tile([DH, 512], F32, tag="bcs", bufs=2)
                        nc.vector.tensor_copy(bcs[:, :], pbc[:, :])
                        nc.vector.tensor_tensor(
                            out=ost[h // 2][(h % 2) * DH:(h % 2 + 1) * DH,
                                            j * 512:(j + 1) * 512],
                            in0=av[j][0:DH, :], in1=bcs[:, :], op=OP.mult)
                    # -ln(den) in place, then transpose to per-partition layout
                    nc.scalar.activation(den_row[:, :], den_row[:, :], AF.Ln)
                    nc.vector.tensor_scalar_mul(den_row[:, :], den_row[:, :], -1.0)
                    nlnd = den_row
                    for i in range(SQ // 128):
                        ptd = psa.tile([128, 1], F32, tag="st", bufs=2, name="ptd")
                        nc.tensor.transpose(ptd[:, :],
                                            nlnd[0:1, i * 128:(i + 1) * 128],
                                            ident[0:1, 0:1])
                        nc.vector.tensor_copy(nld[h][:, i:i + 1], ptd[:, 0:1])

                # ===== phase B (per head pair): natural S, attn output =======
                # Heads 2g/2g+1 live at PE row-groups 0-63/64-127 of the same
                # q/k tiles, so their K=64 matmuls run concurrently and one
                # head's weight load hides under the other's matmul.
                if h % 2 == 1:
                    h0, h1 = h - 1, h
                    with tc.tile_pool(name=f"psB{h}", bufs=2, space="PSUM") as psb:
                        for i in range(SQ // 128):
                            asb0 = apool.tile([128, SKV], F32, tag="a_sb0",
                                              bufs=3, name="asb0")
                            asb1 = apool.tile([128, SKV], F32, tag="a_sb1",
                                              bufs=3, name="asb1")
                            for sh in range(2):
                                pa = psb.tile([128, 1024], F32, tag="spA",
                                              bufs=2, name="pa")
                                pb = psb.tile([128, 1024], F32, tag="spB",
                                              bufs=2, name="pb")
                                for jj in range(2):
                                    col = sh * 1024 + jj * 512
                                    nc.tensor.matmul(
                                        pa[:, jj * 512:(jj + 1) * 512],
                                        qT[h0][:, i * 128:(i + 1) * 128],
                                        kT[h0][:, col:col + 512],
                                        start=True, stop=True)
                                    nc.tensor.matmul(
                                        pb[:, jj * 512:(jj + 1) * 512],
                                        qT[h1][:, i * 128:(i + 1) * 128],
                                        kT[h1][:, col:col + 512],
                                        start=True, stop=True)
                                nc.scalar.activation(
                                    asb0[:, sh * 1024:(sh + 1) * 1024], pa[:, :],
                                    AF.Exp, scale=SCALE, bias=nld[h0][:, i:i + 1])
                                nc.scalar.activation(
                                    asb1[:, sh * 1024:(sh + 1) * 1024], pb[:, :],
                                    AF.Exp, scale=SCALE, bias=nld[h1][:, i:i + 1])
                            nc.sync.dma_start(
                                out=attn_p[h0, i * 128:(i + 1) * 128, :],
                                in_=asb0[:, :])
                            nc.sync.dma_start(
                                out=attn_p[h1, i * 128:(i + 1) * 128, :],
                                in_=asb1[:, :])

            apool_cm.__exit__(None, None, None)

            # ================= out-projection ================================
            with tc.tile_pool(name="ps_o", bufs=2, space="PSUM") as pso:
                for i in range(SQ // 128):
                    po = pso.tile([128, D], F32, tag="po")
                    for c in range(2):
                        nc.tensor.matmul(
                            po[:, :],
                            ost[c][:, i * 128:(i + 1) * 128],
                            wo_r[:, c, :],
                            start=(c == 0), stop=(c == 1))
                    osb = wpool.tile([128, D], F32, tag="osb", bufs=2)
                    nc.vector.tensor_copy(osb[:, :], po[:, :])
                    nc.sync.dma_start(out=out_p[i * 128:(i + 1) * 128, :],
                                      in_=osb[:, :])

    nc.finalize()
    return nc


_NC_CACHE = {}


def _get_nc():
    if "nc" not in _NC_CACHE:
        _NC_CACHE["nc"] = build_kernel()
    return _NC_CACHE["nc"]


def kernel(x, y, ln_q_g, ln_q_b, ln_kv_g, ln_kv_b, Wq, Wk, Wv, Wo, bo,
           _trace=False):
    x = np.ascontiguousarray(np.asarray(x, dtype=np.float32))
    y = np.ascontiguousarray(np.asarray(y, dtype=np.float32))
    Wq = np.asarray(Wq, dtype=np.float32)
    Wk = np.asarray(Wk, dtype=np.float32)
    Wv = np.asarray(Wv, dtype=np.float32)
    Wo = np.asarray(Wo, dtype=np.float32)
    bo = np.asarray(bo, dtype=np.float32)

    nc = _get_nc()

    in_maps = []
    for c in range(N_CORES):
        b = c // 2
        h0 = (c % 2) * HPC
        cols = slice(h0 * DH, (h0 + HPC) * DH)
        in_maps.append({
            "x_b": x[b],
            "y_b": y[b],
            "wq": np.ascontiguousarray(Wq[:, cols]),
            "wk": np.ascontiguousarray(Wk[:, cols]),
            "wv": np.ascontiguousarray(Wv[:, cols]),
            "wo": np.ascontiguousarray(Wo[cols, :]),
            "gq": np.asarray(ln_q_g, np.float32).reshape(1, D),
            "bq": np.asarray(ln_q_b, np.float32).reshape(1, D),
            "gkv": np.asarray(ln_kv_g, np.float32).reshape(1, D),
            "bkv": np.asarray(ln_kv_b, np.float32).reshape(1, D),
        })

    res = run_bass_kernel_spmd(nc, in_maps, list(range(N_CORES)), trace=_trace)

    attn = np.empty((B, H, SQ, SKV), dtype=np.float32)
    out = np.empty((B, SQ, H * DH), dtype=np.float32)
    for b in range(B):
        r0 = res.results[2 * b]
        r1 = res.results[2 * b + 1]
        attn[b, 0:HPC] = r0["attn_p"]
        attn[b, HPC:H] = r1["attn_p"]
        out[b] = r0["out_p"] + r1["out_p"] + bo[None, :]

    if _trace:
        kernel.last_exec_time_ns = res.exec_time_ns
        kernel.last_mean_exec_time_ns = res.mean_exec_time_ns
    return out, attn


# revision 14
# speedup vs baseline: 1.0097x; 1.0097x over previous
"""Cross-attention kernel for Trainium2 (8 NeuronCores).

Problem: B=4, SQ=SKV=2048, D=512, H=8, DH=64 cross-attention with
LayerNorm on both streams, returning (out [B,SQ,D], attn [B,H,SQ,SKV]).

Sharding: core c handles batch b = c//2 and heads [4*(c%2), 4*(c%2)+4).
Each core computes LN + QKV projections for its (batch, head-group),
full attention for its 4 heads (including the 64MB attn-probability
slab), and a partial out-projection. Host sums the two half-head
partials per batch and adds the bias.

Device dataflow per core:
  - LN in natural layout (bn_stats/bn_aggr), PE-transpose to get
    xn^T/yn^T with D on partitions.
  - q^T/k^T [64, 2048] per head and V [2048, 4*65] (with a ones column
    per head) via f32r matmuls.
  - Phase A per head: S^T = k^T.T @ q^T tiles, exp on ScalarE, then
    O^T = [V|1]^T @ exp(S^T) accumulated over Skv. Row 64 of O^T is the
    softmax denominator for free. O^T is normalized per-column via a
    PE-broadcast reciprocal and stored (f32r) for the out-projection.
  - Phase B per head: S = q^T.T @ k^T natural tiles; attention
    probabilities emitted in ONE ScalarE op per row-block:
    A = exp(S*scale - ln d) using the per-partition bias input of the
    activation (d transposed from phase A via PE).
  - Out-projection from the stacked normalized O^T chunks.

All heavy matmuls run in float32r (TF32-like, 4x faster than fp32 on
the PE; ~2e-4 relative error).
"""

import sys

if "/opt/trn_rl_repo" not in sys.path:
    sys.path.insert(0, "/opt/trn_rl_repo")

import numpy as np

import concourse.bacc as bacc
import concourse.mybir as mybir
from concourse.tile import TileContext
from concourse.bass_utils import run_bass_kernel_spmd
from concourse.masks import make_identity

F32 = mybir.dt.float32
F32R = mybir.dt.float32r
AF = mybir.ActivationFunctionType
OP = mybir.AluOpType

B, SQ, SKV, D, H, DH = 4, 2048, 2048, 512, 8, 64
HPC = 4          # heads per core
N_CORES = 8
EPS = 1e-5
SCALE = DH ** -0.5
DC = D // 128    # 4 D-chunks of 128


def build_kernel():
    nc = bacc.Bacc()

    x_b = nc.declare_dram_parameter("x_b", [SQ, D], F32, isOutput=False)
    y_b = nc.declare_dram_parameter("y_b", [SKV, D], F32, isOutput=False)
    wq = nc.declare_dram_parameter("wq", [D, HPC * DH], F32, isOutput=False)
    wk = nc.declare_dram_parameter("wk", [D, HPC * DH], F32, isOutput=False)
    wv = nc.declare_dram_parameter("wv", [D, HPC * DH], F32, isOutput=False)
    wo = nc.declare_dram_parameter("wo", [HPC * DH, D], F32, isOutput=False)
    gq = nc.declare_dram_parameter("gq", [1, D], F32, isOutput=False)
    bq = nc.declare_dram_parameter("bq", [1, D], F32, isOutput=False)
    gkv = nc.declare_dram_parameter("gkv", [1, D], F32, isOutput=False)
    bkv = nc.declare_dram_parameter("bkv", [1, D], F32, isOutput=False)

    attn_p = nc.declare_dram_parameter("attn_p", [HPC, SQ, SKV], F32, isOutput=True)
    out_p = nc.declare_dram_parameter("out_p", [SQ, D], F32, isOutput=True)

    with TileContext(nc) as tc:
        with (
            tc.tile_pool(name="const", bufs=1) as cpool,
            tc.tile_pool(name="persist", bufs=1) as ppool,
            tc.tile_pool(name="work", bufs=3) as wpool,
            tc.tile_pool(name="rows", bufs=1) as rpool,
        ):
            # ---- constants ----
            ident = cpool.tile([128, 128], F32)
            make_identity(nc, ident[:, :])
            epst = cpool.tile([128, 1], F32)
            nc.gpsimd.memset(epst[:, :], EPS)
            ones_row = cpool.tile([1, 128], F32)
            nc.gpsimd.memset(ones_row[:, :], 1.0)
            ones_col = cpool.tile([128, 1], F32)
            nc.gpsimd.memset(ones_col[:, :], 1.0)
            ones_row_r = cpool.tile([1, 128], F32R)
            nc.vector.tensor_copy(ones_row_r[:, :], ones_row[:, :])

            # ---- ln scale/shift broadcast to [128, D] ----
            gb_rows = []
            for idx, t in enumerate((gq, bq, gkv, bkv)):
                row = cpool.tile([1, D], F32, tag=f"gbrow{idx}")
                nc.sync.dma_start(out=row[:, :], in_=t[:, :])
                gb_rows.append(row)
            gbb = []  # Gq, Bq, Gkv, Bkv broadcast tiles
            with tc.tile_pool(name="ps_init", bufs=2, space="PSUM") as psi:
                for idx in range(4):
                    pb = psi.tile([128, D], F32, tag="gb")
                    # fp32 matmul (K=1): broadcast row to 128 partitions
                    nc.tensor.matmul(pb[:, :], ones_row[0:1, :], gb_rows[idx][:, :],
                                     start=True, stop=True)
                    t = cpool.tile([128, D], F32, tag=f"gbb{idx}")
                    nc.vector.tensor_copy(t[:, :], pb[:, :])
                    gbb.append(t)

            # ---- weights ----
            wq_r = ppool.tile([128, DC, HPC * DH], F32R)
            wk_r = ppool.tile([128, DC, HPC * DH], F32R)
            wv_r = ppool.tile([128, DC, HPC * DH], F32R)
            wo_r = ppool.tile([128, 2, D], F32R)
            for (dram, tile_r, nchunk) in ((wq, wq_r, DC), (wk, wk_r, DC),
                                           (wv, wv_r, DC), (wo, wo_r, 2)):
                stage = wpool.tile([128, nchunk, dram.shape[1]], F32, tag="wstage", bufs=1)
                nc.sync.dma_start(
                    out=stage[:, :, :],
                    in_=dram.rearrange("(c p) n -> p c n", p=128))
                nc.vector.tensor_copy(tile_r[:, :, :], stage[:, :, :])

            # ---- LN + transpose helper ----
            def ln_transpose(src_dram, G, Bt, dst_pool, tagbase):
                """LayerNorm src [2048, 512] then PE-transpose into 4 chunk
                tiles [128, 2048] (f32r), D on partitions."""
                chunks = [dst_pool.tile([128, SQ], F32R, tag=f"{tagbase}{c}", name=f"{tagbase}{c}")
                          for c in range(DC)]
                with tc.tile_pool(name=f"ps_{tagbase}", bufs=4, space="PSUM") as pst:
                    for i in range(SQ // 128):
                        t = wpool.tile([128, D], F32, tag="ln_in", bufs=2)
                        nc.sync.dma_start(out=t[:, :],
                                          in_=src_dram[i * 128:(i + 1) * 128, :])
                        st = wpool.tile([128, 6], F32, tag="ln_st")
                        nc.vector.bn_stats(st[:, :], t[:, :])
                        ag = wpool.tile([128, 2], F32, tag="ln_ag")
                        nc.vector.bn_aggr(ag[:, :], st[:, :])
                        std = wpool.tile([128, 1], F32, tag="ln_std")
                        nc.scalar.activation(std[:, :], ag[:, 1:2], AF.Sqrt,
                                             bias=epst[:, 0:1])
                        rstd = wpool.tile([128, 1], F32, tag="ln_rstd")
                        nc.vector.reciprocal(rstd[:, :], std[:, :])
                        xc = wpool.tile([128, D], F32, tag="ln_xc", bufs=2)
                        nc.vector.tensor_scalar(
                            xc[:, :], t[:, :], ag[:, 0:1], rstd[:, :],
                            op0=OP.subtract, op1=OP.mult)
                        xg = wpool.tile([128, D], F32, tag="ln_xg", bufs=2)
                        nc.vector.tensor_tensor(out=xg[:, :], in0=xc[:, :],
                                                in1=G[:, :], op=OP.mult)
                        xn = wpool.tile([128, D], F32, tag="ln_xn", bufs=2)
                        nc.vector.tensor_tensor(out=xn[:, :], in0=xg[:, :],
                                                in1=Bt[:, :], op=OP.add)
                        for c in range(DC):
                            pt = pst.tile([128, 128], F32, tag="tr")
                            nc.tensor.transpose(pt[:, :],
                                                xn[:, c * 128:(c + 1) * 128],
                                                ident[:, :])
                            nc.vector.tensor_copy(
                                chunks[c][:, i * 128:(i + 1) * 128], pt[:, :])
                return chunks

            # ---- stream x: LN -> xT -> q^T ----
            q2T = [ppool.tile([128, SQ], F32R, tag=f"q2T{g}", name=f"q2T{g}")
                   for g in range(HPC // 2)]
            k2T = [ppool.tile([128, SKV], F32R, tag=f"k2T{g}", name=f"k2T{g}")
                   for g in range(HPC // 2)]
            qT = [q2T[h // 2][(h % 2) * DH:(h % 2 + 1) * DH, :] for h in range(HPC)]
            kT = [k2T[h // 2][(h % 2) * DH:(h % 2 + 1) * DH, :] for h in range(HPC)]
            # V plus ones column, per skv-chunk: [128, 16, 4*65]
            vp = ppool.tile([128, SKV // 128, HPC * (DH + 1)], F32R)

            with tc.tile_pool(name="xT_pool", bufs=1) as xpool:
                xT = ln_transpose(x_b, gbb[0], gbb[1], xpool, "xT")
                with tc.tile_pool(name="ps_q", bufs=2, space="PSUM") as psq:
                    for h in range(HPC):
                        for j in range(SQ // 512):
                            pq = psq.tile([DH, 512], F32, tag="pq")
                            for c in range(DC):
                                nc.tensor.matmul(
                                    pq[:, :],
                                    wq_r[:, c, h * DH:(h + 1) * DH],
                                    xT[c][:, j * 512:(j + 1) * 512],
                                    start=(c == 0), stop=(c == DC - 1))
                            nc.vector.tensor_copy(
                                qT[h][:, j * 512:(j + 1) * 512], pq[:, :])

            # ---- stream y: LN -> yT -> k^T, V ----
            with tc.tile_pool(name="yT_pool", bufs=1) as ypool:
                yT = ln_transpose(y_b, gbb[2], gbb[3], ypool, "yT")
                with tc.tile_pool(name="ps_k", bufs=2, space="PSUM") as psk:
                    for h in range(HPC):
                        for j in range(SKV // 512):
                            pk = psk.tile([DH, 512], F32, tag="pk")
                            for c in range(DC):
                                nc.tensor.matmul(
                                    pk[:, :],
                                    wk_r[:, c, h * DH:(h + 1) * DH],
                                    yT[c][:, j * 512:(j + 1) * 512],
                                    start=(c == 0), stop=(c == DC - 1))
                            nc.vector.tensor_copy(
                                kT[h][:, j * 512:(j + 1) * 512], pk[:, :])
                    for m in range(SKV // 128):
                        pv = psk.tile([128, HPC * DH], F32, tag="pv")
                        for c in range(DC):
                            nc.tensor.matmul(
                                pv[:, :],
                                yT[c][:, m * 128:(m + 1) * 128],
                                wv_r[:, c, :],
                                start=(c == 0), stop=(c == DC - 1))
                        for h in range(HPC):
                            nc.vector.tensor_copy(
                                vp[:, m, h * 65:h * 65 + DH],
                                pv[:, h * DH:(h + 1) * DH])
                            nc.vector.tensor_copy(
                                vp[:, m, h * 65 + DH:h * 65 + DH + 1],
                                ones_col[:, :])

            # normalized O^T stacked in two 128-row chunks for the out-proj
            ost = [ppool.tile([128, SQ], F32R, tag=f"ost{c}", name=f"ost{c}") for c in range(2)]
            # per-head, per-row-block -ln(denominator): [128, 16]
            nld = [ppool.tile([128, SQ // 128], F32, tag=f"nld{h}", name=f"nld{h}")
                   for h in range(HPC)]

            n_m = SKV // 128  # 16 skv chunks

            apool_cm = tc.tile_pool(name="attn_work", bufs=2)
            apool = apool_cm.__enter__()
            for h in range(HPC):
                # ================= phase A: S^T, AV, denominators ============
                with tc.tile_pool(name=f"psA{h}", bufs=1, space="PSUM") as psa:
                    av = [psa.tile([65, 512], F32, tag=f"av{j}", name=f"av{j}")
                          for j in range(4)]
                    den_row = rpool.tile([1, SQ], F32, tag="den_row")
                    for half in range(2):
                        for m in range(n_m):
                            ps_st = psa.tile([128, 1024], F32, tag="st", bufs=2)
                            for jj in range(2):
                                nc.tensor.matmul(
                                    ps_st[:, jj * 512:(jj + 1) * 512],
                                    kT[h][:, m * 128:(m + 1) * 128],
                                    qT[h][:, half * 1024 + jj * 512:
                                           half * 1024 + (jj + 1) * 512],
                                    start=True, stop=True)
                            eT = apool.tile([128, 1024], F32R, tag="eT", bufs=3)
                            nc.scalar.activation(eT[:, :], ps_st[:, :], AF.Exp,
                                                 scale=SCALE)
                            for jj in range(2):
                                nc.tensor.matmul(
                                    av[half * 2 + jj][:, :],
                                    vp[:, m, h * 65:(h + 1) * 65],
                                    eT[:, jj * 512:(jj + 1) * 512],
                                    start=(m == 0), stop=(m == n_m - 1))
                    # normalize O^T, collect denominators
                    for j in range(4):
                        nc.vector.tensor_copy(den_row[0:1, j * 512:(j + 1) * 512],
                                              av[j][DH:DH + 1, :])
                        recip = rpool.tile([1, 512], F32, tag="recip")
                        nc.vector.reciprocal(recip[:, :], av[j][DH:DH + 1, :])
                        recip_r = rpool.tile([1, 512], F32R, tag="recip_r")
                        nc.vector.tensor_copy(recip_r[:, :], recip[:, :])
                        pbc = psa.tile([DH, 512], F32, tag="st", bufs=2, name="pbc")
                        nc.tensor.matmul(pbc[:, :], ones_row_r[0:1, 0:DH],
                                         recip_r[:, :], start=True, stop=True)
                        bcs = wpool.tile([DH, 512], F32, tag="bcs", bufs=2)
                        nc.vector.tensor_copy(bcs[:, :], pbc[:, :])
                        nc.vector.tensor_tensor(
                            out=ost[h // 2][(h % 2) * DH:(h % 2 + 1) * DH,
                                            j * 512:(j + 1) * 512],
                            in0=av[j][0:DH, :], in1=bcs[:, :], op=OP.mult)
                    # -ln(den) in place, then transpose to per-partition layout
                    nc.scalar.activation(den_row[:, :], den_row[:, :], AF.Ln)
                    nc.vector.tensor_scalar_mul(den_row[:, :], den_row[:, :], -1.0)
                    nlnd = den_row
                    for i in range(SQ // 128):
                        ptd = psa.tile([128, 1], F32, tag="st", bufs=2, name="ptd")
                        nc.tensor.transpose(ptd[:, :],
                                            nlnd[0:1, i * 128:(i + 1) * 128],
                                            ident[0:1, 0:1])
                        nc.vector.tensor_copy(nld[h][:, i:i + 1], ptd[:, 0:1])

                # ===== phase B (per head pair): natural S, attn output =======
                # Heads 2g/2g+1 live at PE row-groups 0-63/64-127 of the same
                # q/k tiles, so their K=64 matmuls run concurrently and one
                # head's weight load hides under the other's matmul.
                if h % 2 == 1:
                    h0, h1 = h - 1, h
                    with tc.tile_pool(name=f"psB{h}", bufs=2, space="PSUM") as psb:
                        for i in range(SQ // 128):
                            asb0 = apool.tile([128, SKV], F32, tag="a_sb0",
                                              bufs=3, name="asb0")
                            asb1 = apool.tile([128, SKV], F32, tag="a_sb1",
                                              bufs=3, name="asb1")
                            for sh in range(2):
                                pa = psb.tile([128, 1024], F32, tag="spA",
                                              bufs=2, name="pa")
                                pb = psb.tile([128, 1024], F32, tag="spB",
                                              bufs=2, name="pb")
                                for jj in range(2):
                                    col = sh * 1024 + jj * 512
                                    nc.tensor.matmul(
                                        pa[:, jj * 512:(jj + 1) * 512],
                                        qT[h0][:, i * 128:(i + 1) * 128],
                                        kT[h0][:, col:col + 512],
                                        start=True, stop=True,
                                        tile_position=(0, 0))
                                    nc.tensor.matmul(
                                        pb[:, jj * 512:(jj + 1) * 512],
                                        qT[h1][:, i * 128:(i + 1) * 128],
                                        kT[h1][:, col:col + 512],
                                        start=True, stop=True,
                                        tile_position=(64, 0))
                                nc.scalar.activation(
                                    asb0[:, sh * 1024:(sh + 1) * 1024], pa[:, :],
                                    AF.Exp, scale=SCALE, bias=nld[h0][:, i:i + 1])
                                nc.scalar.activation(
                                    asb1[:, sh * 1024:(sh + 1) * 1024], pb[:, :],
                                    AF.Exp, scale=SCALE, bias=nld[h1][:, i:i + 1])
                            nc.sync.dma_start(
                                out=attn_p[h0, i * 128:(i + 1) * 128, :],
                                in_=asb0[:, :])
                            nc.sync.dma_start(
                                out=attn_p[h1, i * 128:(i + 1) * 128, :],
                                in_=asb1[:, :])

            apool_cm.__exit__(None, None, None)

            # ================= out-projection ================================
            with tc.tile_pool(name="ps_o", bufs=2, space="PSUM") as pso:
                for i in range(SQ // 128):
                    po = pso.tile([128, D], F32, tag="po")
                    for c in range(2):
                        nc.tensor.matmul(
                            po[:, :],
                            ost[c][:, i * 128:(i + 1) * 128],
                            wo_r[:, c, :],
                            start=(c == 0), stop=(c == 1))
                    osb = wpool.tile([128, D], F32, tag="osb", bufs=2)
                    nc.vector.tensor_copy(osb[:, :], po[:, :])
                    nc.sync.dma_start(out=out_p[i * 128:(i + 1) * 128, :],
                                      in_=osb[:, :])

    nc.finalize()
    return nc


_NC_CACHE = {}


def _get_nc():
    if "nc" not in _NC_CACHE:
        _NC_CACHE["nc"] = build_kernel()
    return _NC_CACHE["nc"]


def kernel(x, y, ln_q_g, ln_q_b, ln_kv_g, ln_kv_b, Wq, Wk, Wv, Wo, bo,
           _trace=False):
    x = np.ascontiguousarray(np.asarray(x, dtype=np.float32))
    y = np.ascontiguousarray(np.asarray(y, dtype=np.float32))
    Wq = np.asarray(Wq, dtype=np.float32)
    Wk = np.asarray(Wk, dtype=np.float32)
    Wv = np.asarray(Wv, dtype=np.float32)
    Wo = np.asarray(Wo, dtype=np.float32)
    bo = np.asarray(bo, dtype=np.float32)

    nc = _get_nc()

    in_maps = []
    for c in range(N_CORES):
        b = c // 2
        h0 = (c % 2) * HPC
        cols = slice(h0 * DH, (h0 + HPC) * DH)
        in_maps.append({
            "x_b": x[b],
            "y_b": y[b],
            "wq": np.ascontiguousarray(Wq[:, cols]),
            "wk": np.ascontiguousarray(Wk[:, cols]),
            "wv": np.ascontiguousarray(Wv[:, cols]),
            "wo": np.ascontiguousarray(Wo[cols, :]),
            "gq": np.asarray(ln_q_g, np.float32).reshape(1, D),
            "bq": np.asarray(ln_q_b, np.float32).reshape(1, D),
            "gkv": np.asarray(ln_kv_g, np.float32).reshape(1, D),
            "bkv": np.asarray(ln_kv_b, np.float32).reshape(1, D),
        })

    res = run_bass_kernel_spmd(nc, in_maps, list(range(N_CORES)), trace=_trace)

    attn = np.empty((B, H, SQ, SKV), dtype=np.float32)
    out = np.empty((B, SQ, H * DH), dtype=np.float32)
    for b in range(B):
        r0 = res.results[2 * b]
        r1 = res.results[2 * b + 1]
        attn[b, 0:HPC] = r0["attn_p"]
        attn[b, HPC:H] = r1["attn_p"]
        out[b] = r0["out_p"] + r1["out_p"] + bo[None, :]

    if _trace:
        kernel.last_exec_time_ns = res.exec_time_ns
        kernel.last_mean_exec_time_ns = res.mean_exec_time_ns
    return out, attn


# revision 15
# speedup vs baseline: 1.0594x; 1.0493x over previous
"""Cross-attention kernel for Trainium2 (8 NeuronCores).

Problem: B=4, SQ=SKV=2048, D=512, H=8, DH=64 cross-attention with
LayerNorm on both streams, returning (out [B,SQ,D], attn [B,H,SQ,SKV]).

Sharding: core c handles batch b = c//2 and heads [4*(c%2), 4*(c%2)+4).
Each core computes LN + QKV projections for its (batch, head-group),
full attention for its 4 heads (including the 64MB attn-probability
slab), and a partial out-projection. Host sums the two half-head
partials per batch and adds the bias.

Device dataflow per core:
  - LN in natural layout (bn_stats/bn_aggr), PE-transpose to get
    xn^T/yn^T with D on partitions.
  - q^T/k^T [64, 2048] per head and V [2048, 4*65] (with a ones column
    per head) via f32r matmuls.
  - Phase A per head: S^T = k^T.T @ q^T tiles, exp on ScalarE, then
    O^T = [V|1]^T @ exp(S^T) accumulated over Skv. Row 64 of O^T is the
    softmax denominator for free. O^T is normalized per-column via a
    PE-broadcast reciprocal and stored (f32r) for the out-projection.
  - Phase B per head: S = q^T.T @ k^T natural tiles; attention
    probabilities emitted in ONE ScalarE op per row-block:
    A = exp(S*scale - ln d) using the per-partition bias input of the
    activation (d transposed from phase A via PE).
  - Out-projection from the stacked normalized O^T chunks.

All heavy matmuls run in float32r (TF32-like, 4x faster than fp32 on
the PE; ~2e-4 relative error).
"""

import sys

if "/opt/trn_rl_repo" not in sys.path:
    sys.path.insert(0, "/opt/trn_rl_repo")

import numpy as np

import concourse.bacc as bacc
import concourse.mybir as mybir
from concourse.tile import TileContext
from concourse.bass_utils import run_bass_kernel_spmd
from concourse.masks import make_identity

F32 = mybir.dt.float32
F32R = mybir.dt.float32r
AF = mybir.ActivationFunctionType
OP = mybir.AluOpType

B, SQ, SKV, D, H, DH = 4, 2048, 2048, 512, 8, 64
HPC = 4          # heads per core
N_CORES = 8
EPS = 1e-5
SCALE = DH ** -0.5
DC = D // 128    # 4 D-chunks of 128


def build_kernel():
    nc = bacc.Bacc()

    x_b = nc.declare_dram_parameter("x_b", [SQ, D], F32, isOutput=False)
    y_b = nc.declare_dram_parameter("y_b", [SKV, D], F32, isOutput=False)
    wq = nc.declare_dram_parameter("wq", [D, HPC * DH], F32, isOutput=False)
    wk = nc.declare_dram_parameter("wk", [D, HPC * DH], F32, isOutput=False)
    wv = nc.declare_dram_parameter("wv", [D, HPC * DH], F32, isOutput=False)
    wo = nc.declare_dram_parameter("wo", [HPC * DH, D], F32, isOutput=False)
    gq = nc.declare_dram_parameter("gq", [1, D], F32, isOutput=False)
    bq = nc.declare_dram_parameter("bq", [1, D], F32, isOutput=False)
    gkv = nc.declare_dram_parameter("gkv", [1, D], F32, isOutput=False)
    bkv = nc.declare_dram_parameter("bkv", [1, D], F32, isOutput=False)

    attn_p = nc.declare_dram_parameter("attn_p", [HPC, SQ, SKV], F32, isOutput=True)
    out_p = nc.declare_dram_parameter("out_p", [SQ, D], F32, isOutput=True)

    with TileContext(nc) as tc:
        with (
            tc.tile_pool(name="const", bufs=1) as cpool,
            tc.tile_pool(name="persist", bufs=1) as ppool,
            tc.tile_pool(name="work", bufs=3) as wpool,
            tc.tile_pool(name="rows", bufs=1) as rpool,
        ):
            # ---- constants ----
            ident = cpool.tile([128, 128], F32)
            make_identity(nc, ident[:, :])
            epst = cpool.tile([128, 1], F32)
            nc.gpsimd.memset(epst[:, :], EPS)
            ones_row = cpool.tile([1, 128], F32)
            nc.gpsimd.memset(ones_row[:, :], 1.0)
            ones_col = cpool.tile([128, 1], F32)
            nc.gpsimd.memset(ones_col[:, :], 1.0)
            ones_row_r = cpool.tile([1, 128], F32R)
            nc.vector.tensor_copy(ones_row_r[:, :], ones_row[:, :])

            # ---- ln scale/shift broadcast to [128, D] ----
            gb_rows = []
            for idx, t in enumerate((gq, bq, gkv, bkv)):
                row = cpool.tile([1, D], F32, tag=f"gbrow{idx}")
                nc.sync.dma_start(out=row[:, :], in_=t[:, :])
                gb_rows.append(row)
            gbb = []  # Gq, Bq, Gkv, Bkv broadcast tiles
            with tc.tile_pool(name="ps_init", bufs=2, space="PSUM") as psi:
                for idx in range(4):
                    pb = psi.tile([128, D], F32, tag="gb")
                    # fp32 matmul (K=1): broadcast row to 128 partitions
                    nc.tensor.matmul(pb[:, :], ones_row[0:1, :], gb_rows[idx][:, :],
                                     start=True, stop=True)
                    t = cpool.tile([128, D], F32, tag=f"gbb{idx}")
                    nc.vector.tensor_copy(t[:, :], pb[:, :])
                    gbb.append(t)

            # ---- weights ----
            wq_r = ppool.tile([128, DC, HPC * DH], F32R)
            wk_r = ppool.tile([128, DC, HPC * DH], F32R)
            wv_r = ppool.tile([128, DC, HPC * DH], F32R)
            wo_r = ppool.tile([128, 2, D], F32R)
            for (dram, tile_r, nchunk) in ((wq, wq_r, DC), (wk, wk_r, DC),
                                           (wv, wv_r, DC), (wo, wo_r, 2)):
                stage = wpool.tile([128, nchunk, dram.shape[1]], F32, tag="wstage", bufs=1)
                nc.sync.dma_start(
                    out=stage[:, :, :],
                    in_=dram.rearrange("(c p) n -> p c n", p=128))
                nc.vector.tensor_copy(tile_r[:, :, :], stage[:, :, :])

            # ---- LN + transpose helper ----
            def ln_transpose(src_dram, G, Bt, dst_pool, tagbase):
                """LayerNorm src [2048, 512] then PE-transpose into 4 chunk
                tiles [128, 2048] (f32r), D on partitions."""
                chunks = [dst_pool.tile([128, SQ], F32R, tag=f"{tagbase}{c}", name=f"{tagbase}{c}")
                          for c in range(DC)]
                with tc.tile_pool(name=f"ps_{tagbase}", bufs=4, space="PSUM") as pst:
                    for i in range(SQ // 128):
                        t = wpool.tile([128, D], F32, tag="ln_in", bufs=2)
                        nc.sync.dma_start(out=t[:, :],
                                          in_=src_dram[i * 128:(i + 1) * 128, :])
                        st = wpool.tile([128, 6], F32, tag="ln_st")
                        nc.vector.bn_stats(st[:, :], t[:, :])
                        ag = wpool.tile([128, 2], F32, tag="ln_ag")
                        nc.vector.bn_aggr(ag[:, :], st[:, :])
                        std = wpool.tile([128, 1], F32, tag="ln_std")
                        nc.scalar.activation(std[:, :], ag[:, 1:2], AF.Sqrt,
                                             bias=epst[:, 0:1])
                        rstd = wpool.tile([128, 1], F32, tag="ln_rstd")
                        nc.vector.reciprocal(rstd[:, :], std[:, :])
                        xc = wpool.tile([128, D], F32, tag="ln_xc", bufs=2)
                        nc.vector.tensor_scalar(
                            xc[:, :], t[:, :], ag[:, 0:1], rstd[:, :],
                            op0=OP.subtract, op1=OP.mult)
                        xg = wpool.tile([128, D], F32, tag="ln_xg", bufs=2)
                        nc.vector.tensor_tensor(out=xg[:, :], in0=xc[:, :],
                                                in1=G[:, :], op=OP.mult)
                        xn = wpool.tile([128, D], F32, tag="ln_xn", bufs=2)
                        nc.vector.tensor_tensor(out=xn[:, :], in0=xg[:, :],
                                                in1=Bt[:, :], op=OP.add)
                        for c in range(DC):
                            pt = pst.tile([128, 128], F32, tag="tr")
                            nc.tensor.transpose(pt[:, :],
                                                xn[:, c * 128:(c + 1) * 128],
                                                ident[:, :])
                            nc.vector.tensor_copy(
                                chunks[c][:, i * 128:(i + 1) * 128], pt[:, :])
                return chunks

            # ---- stream x: LN -> xT -> q^T ----
            q2T = [ppool.tile([128, SQ], F32R, tag=f"q2T{g}", name=f"q2T{g}")
                   for g in range(HPC // 2)]
            k2T = [ppool.tile([128, SKV], F32R, tag=f"k2T{g}", name=f"k2T{g}")
                   for g in range(HPC // 2)]
            qT = [q2T[h // 2][(h % 2) * DH:(h % 2 + 1) * DH, :] for h in range(HPC)]
            kT = [k2T[h // 2][(h % 2) * DH:(h % 2 + 1) * DH, :] for h in range(HPC)]
            # V plus ones column, per skv-chunk: [128, 16, 4*65]
            vp = ppool.tile([128, SKV // 128, HPC * (DH + 1)], F32R)

            with tc.tile_pool(name="xT_pool", bufs=1) as xpool:
                xT = ln_transpose(x_b, gbb[0], gbb[1], xpool, "xT")
                with tc.tile_pool(name="ps_q", bufs=2, space="PSUM") as psq:
                    for h in range(HPC):
                        for j in range(SQ // 512):
                            pq = psq.tile([DH, 512], F32, tag="pq")
                            for c in range(DC):
                                nc.tensor.matmul(
                                    pq[:, :],
                                    wq_r[:, c, h * DH:(h + 1) * DH],
                                    xT[c][:, j * 512:(j + 1) * 512],
                                    start=(c == 0), stop=(c == DC - 1))
                            nc.vector.tensor_copy(
                                qT[h][:, j * 512:(j + 1) * 512], pq[:, :])

            # ---- stream y: LN -> yT -> k^T, V ----
            with tc.tile_pool(name="yT_pool", bufs=1) as ypool:
                yT = ln_transpose(y_b, gbb[2], gbb[3], ypool, "yT")
                with tc.tile_pool(name="ps_k", bufs=2, space="PSUM") as psk:
                    for h in range(HPC):
                        for j in range(SKV // 512):
                            pk = psk.tile([DH, 512], F32, tag="pk")
                            for c in range(DC):
                                nc.tensor.matmul(
                                    pk[:, :],
                                    wk_r[:, c, h * DH:(h + 1) * DH],
                                    yT[c][:, j * 512:(j + 1) * 512],
                                    start=(c == 0), stop=(c == DC - 1))
                            nc.vector.tensor_copy(
                                kT[h][:, j * 512:(j + 1) * 512], pk[:, :])
                    for m in range(SKV // 128):
                        pv = psk.tile([128, HPC * DH], F32, tag="pv")
                        for c in range(DC):
                            nc.tensor.matmul(
                                pv[:, :],
                                yT[c][:, m * 128:(m + 1) * 128],
                                wv_r[:, c, :],
                                start=(c == 0), stop=(c == DC - 1))
                        for h in range(HPC):
                            nc.vector.tensor_copy(
                                vp[:, m, h * 65:h * 65 + DH],
                                pv[:, h * DH:(h + 1) * DH])
                            nc.vector.tensor_copy(
                                vp[:, m, h * 65 + DH:h * 65 + DH + 1],
                                ones_col[:, :])

            # normalized O^T stacked in two 128-row chunks for the out-proj
            ost = [ppool.tile([128, SQ], F32R, tag=f"ost{c}", name=f"ost{c}") for c in range(2)]
            # per-head, per-row-block -ln(denominator): [128, 16]
            nld = [ppool.tile([128, SQ // 128], F32, tag=f"nld{h}", name=f"nld{h}")
                   for h in range(HPC)]

            n_m = SKV // 128  # 16 skv chunks

            apool_cm = tc.tile_pool(name="attn_work", bufs=2)
            apool = apool_cm.__enter__()
            for h in range(HPC):
                # ================= phase A: S^T, AV, denominators ============
                with tc.tile_pool(name=f"psA{h}", bufs=1, space="PSUM") as psa:
                    av = [psa.tile([65, 512], F32, tag=f"av{j}", name=f"av{j}")
                          for j in range(4)]
                    den_row = rpool.tile([1, SQ], F32, tag="den_row")
                    for half in range(2):
                        ets = [None] * n_m
                        for m in range(n_m + 1):
                            if m < n_m:
                                ps_st = psa.tile([128, 1024], F32, tag="st",
                                                 bufs=2, name="ps_st")
                                for jj in range(2):
                                    nc.tensor.matmul(
                                        ps_st[:, jj * 512:(jj + 1) * 512],
                                        kT[h][:, m * 128:(m + 1) * 128],
                                        qT[h][:, half * 1024 + jj * 512:
                                               half * 1024 + (jj + 1) * 512],
                                        start=True, stop=True)
                                ets[m] = apool.tile([128, 1024], F32R,
                                                    tag="eT", bufs=3, name="eT")
                                nc.scalar.activation(ets[m][:, :], ps_st[:, :],
                                                     AF.Exp, scale=SCALE)
                            if m > 0:
                                for jj in range(2):
                                    nc.tensor.matmul(
                                        av[half * 2 + jj][:, :],
                                        vp[:, m - 1, h * 65:(h + 1) * 65],
                                        ets[m - 1][:, jj * 512:(jj + 1) * 512],
                                        start=(m - 1 == 0),
                                        stop=(m - 1 == n_m - 1))
                    # normalize O^T, collect denominators
                    for j in range(4):
                        nc.vector.tensor_copy(den_row[0:1, j * 512:(j + 1) * 512],
                                              av[j][DH:DH + 1, :])
                        recip = rpool.tile([1, 512], F32, tag="recip")
                        nc.vector.reciprocal(recip[:, :], av[j][DH:DH + 1, :])
                        recip_r = rpool.tile([1, 512], F32R, tag="recip_r")
                        nc.vector.tensor_copy(recip_r[:, :], recip[:, :])
                        pbc = psa.tile([DH, 512], F32, tag="st", bufs=2, name="pbc")
                        nc.tensor.matmul(pbc[:, :], ones_row_r[0:1, 0:DH],
                                         recip_r[:, :], start=True, stop=True)
                        bcs = wpool.tile([DH, 512], F32, tag="bcs", bufs=2)
                        nc.vector.tensor_copy(bcs[:, :], pbc[:, :])
                        nc.vector.tensor_tensor(
                            out=ost[h // 2][(h % 2) * DH:(h % 2 + 1) * DH,
                                            j * 512:(j + 1) * 512],
                            in0=av[j][0:DH, :], in1=bcs[:, :], op=OP.mult)
                    # -ln(den) in place, then transpose to per-partition layout
                    nc.scalar.activation(den_row[:, :], den_row[:, :], AF.Ln)
                    nc.vector.tensor_scalar_mul(den_row[:, :], den_row[:, :], -1.0)
                    nlnd = den_row
                    for i in range(SQ // 128):
                        ptd = psa.tile([128, 1], F32, tag="st", bufs=2, name="ptd")
                        nc.tensor.transpose(ptd[:, :],
                                            nlnd[0:1, i * 128:(i + 1) * 128],
                                            ident[0:1, 0:1])
                        nc.vector.tensor_copy(nld[h][:, i:i + 1], ptd[:, 0:1])

                # ================= phase B: natural S, attn output ===========
                with tc.tile_pool(name=f"psB{h}", bufs=2, space="PSUM") as psb:
                    for i in range(SQ // 128):
                        ps_s = psb.tile([128, SKV], F32, tag="sp")
                        for jj in range(SKV // 512):
                            nc.tensor.matmul(
                                ps_s[:, jj * 512:(jj + 1) * 512],
                                qT[h][:, i * 128:(i + 1) * 128],
                                kT[h][:, jj * 512:(jj + 1) * 512],
                                start=True, stop=True)
                        a_sb = apool.tile([128, SKV], F32, tag="a_sb", bufs=6)
                        nc.scalar.activation(a_sb[:, :], ps_s[:, :], AF.Exp,
                                             scale=SCALE, bias=nld[h][:, i:i + 1])
                        nc.sync.dma_start(
                            out=attn_p[h, i * 128:(i + 1) * 128, :],
                            in_=a_sb[:, :])

            apool_cm.__exit__(None, None, None)

            # ================= out-projection ================================
            with tc.tile_pool(name="ps_o", bufs=2, space="PSUM") as pso:
                for i in range(SQ // 128):
                    po = pso.tile([128, D], F32, tag="po")
                    for c in range(2):
                        nc.tensor.matmul(
                            po[:, :],
                            ost[c][:, i * 128:(i + 1) * 128],
                            wo_r[:, c, :],
                            start=(c == 0), stop=(c == 1))
                    osb = wpool.tile([128, D], F32, tag="osb", bufs=2)
                    nc.vector.tensor_copy(osb[:, :], po[:, :])
                    nc.sync.dma_start(out=out_p[i * 128:(i + 1) * 128, :],
                                      in_=osb[:, :])

    nc.finalize()
    return nc


_NC_CACHE = {}


def _get_nc():
    if "nc" not in _NC_CACHE:
        _NC_CACHE["nc"] = build_kernel()
    return _NC_CACHE["nc"]


def kernel(x, y, ln_q_g, ln_q_b, ln_kv_g, ln_kv_b, Wq, Wk, Wv, Wo, bo,
           _trace=False):
    x = np.ascontiguousarray(np.asarray(x, dtype=np.float32))
    y = np.ascontiguousarray(np.asarray(y, dtype=np.float32))
    Wq = np.asarray(Wq, dtype=np.float32)
    Wk = np.asarray(Wk, dtype=np.float32)
    Wv = np.asarray(Wv, dtype=np.float32)
    Wo = np.asarray(Wo, dtype=np.float32)
    bo = np.asarray(bo, dtype=np.float32)

    nc = _get_nc()

    in_maps = []
    for c in range(N_CORES):
        b = c // 2
        h0 = (c % 2) * HPC
        cols = slice(h0 * DH, (h0 + HPC) * DH)
        in_maps.append({
            "x_b": x[b],
            "y_b": y[b],
            "wq": np.ascontiguousarray(Wq[:, cols]),
            "wk": np.ascontiguousarray(Wk[:, cols]),
            "wv": np.ascontiguousarray(Wv[:, cols]),
            "wo": np.ascontiguousarray(Wo[cols, :]),
            "gq": np.asarray(ln_q_g, np.float32).reshape(1, D),
            "bq": np.asarray(ln_q_b, np.float32).reshape(1, D),
            "gkv": np.asarray(ln_kv_g, np.float32).reshape(1, D),
            "bkv": np.asarray(ln_kv_b, np.float32).reshape(1, D),
        })

    res = run_bass_kernel_spmd(nc, in_maps, list(range(N_CORES)), trace=_trace)

    attn = np.empty((B, H, SQ, SKV), dtype=np.float32)
    out = np.empty((B, SQ, H * DH), dtype=np.float32)
    for b in range(B):
        r0 = res.results[2 * b]
        r1 = res.results[2 * b + 1]
        attn[b, 0:HPC] = r0["attn_p"]
        attn[b, HPC:H] = r1["attn_p"]
        out[b] = r0["out_p"] + r1["out_p"] + bo[None, :]

    if _trace:
        kernel.last_exec_time_ns = res.exec_time_ns
        kernel.last_mean_exec_time_ns = res.mean_exec_time_ns
    return out, attn
